# revision 13
# baseline (speedup 1.0000x reference)
"""KL-divergence loss kernel (C51 categorical projection + batchmean KL) for TRN2.

Math: the reference projects `anchor` through a C51 projection whose skew is a
compile-time scalar, so the projection collapses to a constant linear map:

    t[:, 0]  = 0
    t[:, 1]  = 0.75*a[:, 0]
    t[:, j]  = 0.75*a[:, j-1] + 0.25*a[:, j-2]          (2 <= j <= 49)
    t[:, 50] = 0.25*a[:, 48] + a[:, 49] + a[:, 50]

and the loss is sum(t * (log t - log(f + 1e-16))) / B  (terms with t==0 are 0).

Kernel strategy (pure data parallel over 8 cores, batch-sharded):
  s = 4t built with one wide fused scalar_tensor_tensor (s_j = 3*a_{j-1} + a_{j-2})
  lt = Ln(0.25*s + 1e-35)      [ScalarE, fused affine, bf16 out]
  lf = Ln(f + 1e-16)           [ScalarE, fused affine, bf16 out]
  lt and lf land in two halves of one fused SBUF tile; TensorE matmuls
  lhsT=s_blk against rhs spanning BOTH halves (free dim 2*w) so one PSUM
  accumulator [128, 256] collects sum(s*lt) on diag cells [j, j] and
  sum(s*lf) on [j, 128+j].  Host sums diag(lt-half) - diag(lf-half) over
  the 8 per-core results and scales by 0.25/B.  No Vector-engine subtract,
  so the per-tile V->S->V dependency chain of the naive pipeline is gone
  and DMA stays saturated.  The last tile's feature DMA + Ln are split in
  half so only ~1us of ScalarE work remains after the final transfer lands.
"""

import os
import numpy as np

B_TOTAL = 524288
ATOMS = 51
N_CORES = 8
ROWS_PER_CORE = B_TOTAL // N_CORES  # 65536
P = 128
R_SCHED = [64, 64, 64, 64, 64, 64, 64, 32, 32]
assert sum(R_SCHED) * P == ROWS_PER_CORE
MM_BLOCK = 128

_BUILT = None
_LAST_RESULTS = None


def _blocks(cols, edges):
    """128-wide matmul blocks, additionally cut at the given column edges."""
    cuts = sorted(set([0, cols] + [e for e in edges if 0 < e < cols]))
    out = []
    for lo, hi in zip(cuts[:-1], cuts[1:]):
        c = lo
        while c < hi:
            w = min(MM_BLOCK, hi - c)
            out.append((c, w))
            c += w
    return out


def _build():
    from contextlib import ExitStack

    import concourse.bacc as bacc
    import concourse.tile as tile
    from concourse import mybir

    nc = bacc.Bacc("TRN2", num_devices=N_CORES)

    a_dram = nc.dram_tensor(
        "anchor", [ROWS_PER_CORE, ATOMS], mybir.dt.float32, kind="ExternalInput"
    )
    f_dram = nc.dram_tensor(
        "feature", [ROWS_PER_CORE, ATOMS], mybir.dt.float32, kind="ExternalInput"
    )
    out_dram = nc.dram_tensor(
        "out", [P, 2 * MM_BLOCK], mybir.dt.float32, kind="ExternalOutput"
    )

    mult = mybir.AluOpType.mult
    add = mybir.AluOpType.add

    n_tiles = len(R_SCHED)
    last = n_tiles - 1

    total_mms = 0
    for i, R in enumerate(R_SCHED):
        cols = R * ATOMS
        edges = [cols // 2] if i == last else []
        total_mms += len(_blocks(cols, edges))

    with tile.TileContext(nc) as tc:
        with ExitStack() as ctx:
            a_pool = ctx.enter_context(tc.tile_pool(name="a", bufs=3))
            f_pool = ctx.enter_context(tc.tile_pool(name="f", bufs=3))
            s_pool = ctx.enter_context(tc.tile_pool(name="s", bufs=3))
            ll_pool = ctx.enter_context(tc.tile_pool(name="ll", bufs=3))
            tmp_pool = ctx.enter_context(tc.tile_pool(name="tmp", bufs=3))
            out_pool = ctx.enter_context(tc.tile_pool(name="outp", bufs=1))
            psum_pool = ctx.enter_context(
                tc.tile_pool(name="acc", bufs=1, space="PSUM")
            )

            acc = psum_pool.tile([P, 2 * MM_BLOCK], mybir.dt.float32)
            acc2 = acc[:].rearrange("m (two c) -> m two c", two=2)

            eps_t = out_pool.tile([P, 1], mybir.dt.float32, tag="eps_t")
            eps_f = out_pool.tile([P, 1], mybir.dt.float32, tag="eps_f")
            warm = out_pool.tile([P, 1], mybir.dt.float32, tag="warm")
            nc.gpsimd.memset(eps_t[:], 1e-35)
            nc.gpsimd.memset(eps_f[:], 1e-16)
            # dummy activation: hoists the ~1.3us ACT_TABLE_LOAD off the
            # critical path (otherwise it lands right before the first real Ln)
            nc.scalar.activation(
                out=warm[:],
                in_=eps_f[:],
                func=mybir.ActivationFunctionType.Ln,
                bias=eps_f[:],
                scale=1.0,
            )

            mm = 0
            r0 = 0
            for i, R in enumerate(R_SCHED):
                cols = R * ATOMS
                a_t = (
                    a_dram.ap()[r0 : r0 + P * R, :]
                    .rearrange("(p q) m -> p (q m)", p=P)
                )
                f_t3 = (
                    f_dram.ap()[r0 : r0 + P * R, :]
                    .rearrange("(p q) m -> p q m", p=P)
                )
                r0 += P * R

                a_sb = a_pool.tile([P, cols], mybir.dt.float32)
                f_sb = f_pool.tile([P, cols], mybir.dt.float32)
                nc.sync.dma_start(out=a_sb[:], in_=a_t)
                if i == last:
                    # split the final feature transfer so Ln(f) can start on
                    # the first half while the second half is still landing
                    h = R // 2
                    nc.sync.dma_start(
                        out=f_sb[:, 0 : h * ATOMS], in_=f_t3[:, 0:h, :]
                    )
                    nc.sync.dma_start(
                        out=f_sb[:, h * ATOMS : cols], in_=f_t3[:, h:R, :]
                    )
                else:
                    nc.sync.dma_start(
                        out=f_sb[:], in_=f_t3.rearrange("p q m -> p (q m)")
                    )

                s_sb = s_pool.tile([P, cols], mybir.dt.bfloat16)
                ll_sb = ll_pool.tile([P, 2 * cols], mybir.dt.bfloat16)
                tmp = tmp_pool.tile([P, R], mybir.dt.float32)

                a3 = a_sb[:].rearrange("p (q m) -> p q m", m=ATOMS)
                s3 = s_sb[:].rearrange("p (q m) -> p q m", m=ATOMS)

                nc.gpsimd.memset(s3[:, :, 0], 0.0)
                # s_j = 3*a_{j-1} + a_{j-2} for j in 2..49
                nc.vector.scalar_tensor_tensor(
                    out=s3[:, :, 2:50],
                    in0=a3[:, :, 1:49],
                    scalar=3.0,
                    in1=a3[:, :, 0:48],
                    op0=mult,
                    op1=add,
                )
                # s_1 = 3*a_0
                nc.vector.tensor_scalar_mul(s3[:, :, 1], a3[:, :, 0], 3.0)
                # s_50 = a_48 + 4*a_49 + 4*a_50
                nc.vector.scalar_tensor_tensor(
                    out=tmp[:],
                    in0=a3[:, :, 49],
                    scalar=4.0,
                    in1=a3[:, :, 48],
                    op0=mult,
                    op1=add,
                )
                nc.vector.scalar_tensor_tensor(
                    out=s3[:, :, 50],
                    in0=a3[:, :, 50],
                    scalar=4.0,
                    in1=tmp[:],
                    op0=mult,
                    op1=add,
                )

                # lf = Ln(f + 1e-16) first (f just landed; s's chain is
                # longer), then lt = Ln(0.25*s + 1e-35)
                if i == last:
                    hc = (R // 2) * ATOMS
                    nc.scalar.activation(
                        out=ll_sb[:, cols : cols + hc],
                        in_=f_sb[:, 0:hc],
                        func=mybir.ActivationFunctionType.Ln,
                        bias=eps_f[:],
                        scale=1.0,
                    )
                    nc.scalar.activation(
                        out=ll_sb[:, 0:cols],
                        in_=s_sb[:],
                        func=mybir.ActivationFunctionType.Ln,
                        bias=eps_t[:],
                        scale=0.25,
                    )
                    nc.scalar.activation(
                        out=ll_sb[:, cols + hc : 2 * cols],
                        in_=f_sb[:, hc:cols],
                        func=mybir.ActivationFunctionType.Ln,
                        bias=eps_f[:],
                        scale=1.0,
                    )
                else:
                    nc.scalar.activation(
                        out=ll_sb[:, cols : 2 * cols],
                        in_=f_sb[:],
                        func=mybir.ActivationFunctionType.Ln,
                        bias=eps_f[:],
                        scale=1.0,
                    )
                    nc.scalar.activation(
                        out=ll_sb[:, 0:cols],
                        in_=s_sb[:],
                        func=mybir.ActivationFunctionType.Ln,
                        bias=eps_t[:],
                        scale=0.25,
                    )

                ll2 = ll_sb[:].rearrange("p (two c) -> p two c", two=2)
                edges = [cols // 2] if i == last else []
                for c0, w in _blocks(cols, edges):
                    nc.tensor.matmul(
                        acc2[0:w, :, 0:w],
                        s_sb[:, c0 : c0 + w],
                        ll2[:, :, c0 : c0 + w],
                        start=(mm == 0),
                        stop=(mm == total_mms - 1),
                    )
                    mm += 1

            out_sb = out_pool.tile([P, 2 * MM_BLOCK], mybir.dt.float32)
            nc.vector.tensor_copy(out_sb[:], acc[:])
            nc.sync.dma_start(out=out_dram.ap(), in_=out_sb[:])

    nc.compile()
    return nc


def kernel(anchor: np.ndarray, feature: np.ndarray) -> np.ndarray:
    global _BUILT, _LAST_RESULTS
    from concourse import bass_utils

    if _BUILT is None:
        _BUILT = _build()
    nc = _BUILT

    anchor = np.ascontiguousarray(anchor, dtype=np.float32)
    feature = np.ascontiguousarray(feature, dtype=np.float32)

    in_maps = []
    for c in range(N_CORES):
        lo, hi = c * ROWS_PER_CORE, (c + 1) * ROWS_PER_CORE
        in_maps.append({"anchor": anchor[lo:hi], "feature": feature[lo:hi]})

    res = bass_utils.run_bass_kernel_spmd(
        nc,
        in_maps,
        core_ids=list(range(N_CORES)),
        trace=bool(os.environ.get("BASS_TRACE")),
    )
    _LAST_RESULTS = res

    total = 0.0
    for c in range(N_CORES):
        out = res.results[c]["out"].astype(np.float64)
        total += np.trace(out[:, :MM_BLOCK]) - np.trace(out[:, MM_BLOCK:])
    val = 0.25 * total / B_TOTAL
    return np.array(val, dtype=np.float32)


# revision 14
# speedup vs baseline: 1.1635x; 1.1635x over previous
"""KL-divergence loss kernel (C51 categorical projection + batchmean KL) for TRN2.

Math: the reference projects `anchor` through a C51 projection whose skew is a
compile-time scalar, so the projection collapses to a constant linear map:

    t[:, 0]  = 0
    t[:, 1]  = 0.75*a[:, 0]
    t[:, j]  = 0.75*a[:, j-1] + 0.25*a[:, j-2]          (2 <= j <= 49)
    t[:, 50] = 0.25*a[:, 48] + a[:, 49] + a[:, 50]

and the loss is sum(t * (log t - log(f + 1e-16))) / B  (terms with t==0 are 0).

Kernel strategy (pure data parallel over 8 cores, batch-sharded):
  s = 4t built with one wide fused scalar_tensor_tensor (s_j = 3*a_{j-1} + a_{j-2})
  lt = Ln(0.25*s + 1e-35)      [ScalarE, fused affine, bf16 out]
  lf = Ln(f + 1e-16)           [ScalarE, fused affine, bf16 out]
  lt and lf land in two halves of one fused SBUF tile; TensorE matmuls
  lhsT=s_blk against rhs spanning BOTH halves (free dim 2*w) so one PSUM
  accumulator [128, 256] collects sum(s*lt) on diag cells [j, j] and
  sum(s*lf) on [j, 128+j].  Host sums diag(lt-half) - diag(lf-half) over
  the 8 per-core results and scales by 0.25/B.  No Vector-engine subtract,
  so the per-tile V->S->V dependency chain of the naive pipeline is gone
  and DMA stays saturated.  The last tile's feature DMA + Ln are split in
  half so only ~1us of ScalarE work remains after the final transfer lands.
"""

import os
import numpy as np

B_TOTAL = 524288
ATOMS = 51
N_CORES = 8
ROWS_PER_CORE = B_TOTAL // N_CORES  # 65536
P = 128
R_SCHED = [64, 64, 64, 64, 64, 64, 64, 32, 32]
assert sum(R_SCHED) * P == ROWS_PER_CORE
MM_BLOCK = 128

_BUILT = None
_LAST_RESULTS = None


def _blocks(cols, edges):
    """128-wide matmul blocks, additionally cut at the given column edges."""
    cuts = sorted(set([0, cols] + [e for e in edges if 0 < e < cols]))
    out = []
    for lo, hi in zip(cuts[:-1], cuts[1:]):
        c = lo
        while c < hi:
            w = min(MM_BLOCK, hi - c)
            out.append((c, w))
            c += w
    return out


def _build():
    from contextlib import ExitStack

    import concourse.bacc as bacc
    import concourse.tile as tile
    from concourse import mybir

    nc = bacc.Bacc("TRN2", num_devices=N_CORES)

    a_dram = nc.dram_tensor(
        "anchor", [ROWS_PER_CORE, ATOMS], mybir.dt.float32, kind="ExternalInput"
    )
    f_dram = nc.dram_tensor(
        "feature", [ROWS_PER_CORE, ATOMS], mybir.dt.float32, kind="ExternalInput"
    )
    out_dram = nc.dram_tensor(
        "out", [P, 2 * MM_BLOCK], mybir.dt.float32, kind="ExternalOutput"
    )

    mult = mybir.AluOpType.mult
    add = mybir.AluOpType.add

    n_tiles = len(R_SCHED)
    last = n_tiles - 1

    total_mms = 0
    for i, R in enumerate(R_SCHED):
        cols = R * ATOMS
        edges = [cols // 2] if i == last else []
        total_mms += len(_blocks(cols, edges))

    with tile.TileContext(nc) as tc:
        with ExitStack() as ctx:
            a_pool = ctx.enter_context(tc.tile_pool(name="a", bufs=3))
            f_pool = ctx.enter_context(tc.tile_pool(name="f", bufs=3))
            s_pool = ctx.enter_context(tc.tile_pool(name="s", bufs=3))
            ll_pool = ctx.enter_context(tc.tile_pool(name="ll", bufs=3))
            tmp_pool = ctx.enter_context(tc.tile_pool(name="tmp", bufs=3))
            out_pool = ctx.enter_context(tc.tile_pool(name="outp", bufs=1))
            psum_pool = ctx.enter_context(
                tc.tile_pool(name="acc", bufs=1, space="PSUM")
            )

            acc = psum_pool.tile([P, 2 * MM_BLOCK], mybir.dt.float32)
            acc2 = acc[:].rearrange("m (two c) -> m two c", two=2)

            eps_t = out_pool.tile([P, 1], mybir.dt.float32, tag="eps_t")
            eps_f = out_pool.tile([P, 1], mybir.dt.float32, tag="eps_f")
            warm = out_pool.tile([P, 1], mybir.dt.float32, tag="warm")
            nc.gpsimd.memset(eps_t[:], 1e-35)
            nc.gpsimd.memset(eps_f[:], 1e-16)
            # dummy activation: hoists the ~1.3us ACT_TABLE_LOAD off the
            # critical path (otherwise it lands right before the first real Ln)
            nc.scalar.activation(
                out=warm[:],
                in_=eps_f[:],
                func=mybir.ActivationFunctionType.Ln,
                bias=eps_f[:],
                scale=1.0,
            )

            mm = 0
            r0 = 0
            for i, R in enumerate(R_SCHED):
                cols = R * ATOMS
                a_t = (
                    a_dram.ap()[r0 : r0 + P * R, :]
                    .rearrange("(p q) m -> p (q m)", p=P)
                )
                f_t3 = (
                    f_dram.ap()[r0 : r0 + P * R, :]
                    .rearrange("(p q) m -> p q m", p=P)
                )
                r0 += P * R

                a_sb = a_pool.tile([P, cols], mybir.dt.float32)
                f_sb = f_pool.tile([P, cols], mybir.dt.float32)
                nc.sync.dma_start(out=a_sb[:], in_=a_t)
                if i == last:
                    # split the final feature transfer so Ln(f) can start on
                    # the first half while the second half is still landing
                    h = R // 2
                    nc.sync.dma_start(
                        out=f_sb[:, 0 : h * ATOMS], in_=f_t3[:, 0:h, :]
                    )
                    nc.sync.dma_start(
                        out=f_sb[:, h * ATOMS : cols], in_=f_t3[:, h:R, :]
                    )
                else:
                    nc.sync.dma_start(
                        out=f_sb[:], in_=f_t3.rearrange("p q m -> p (q m)")
                    )

                s_sb = s_pool.tile([P, cols], mybir.dt.bfloat16)
                ll_sb = ll_pool.tile([P, 2 * cols], mybir.dt.bfloat16)
                tmp = tmp_pool.tile([P, R], mybir.dt.float32)

                a3 = a_sb[:].rearrange("p (q m) -> p q m", m=ATOMS)
                s3 = s_sb[:].rearrange("p (q m) -> p q m", m=ATOMS)

                nc.gpsimd.memset(s3[:, :, 0], 0.0)
                # s_j = 3*a_{j-1} + a_{j-2} for j in 2..49
                nc.vector.scalar_tensor_tensor(
                    out=s3[:, :, 2:50],
                    in0=a3[:, :, 1:49],
                    scalar=3.0,
                    in1=a3[:, :, 0:48],
                    op0=mult,
                    op1=add,
                )
                # s_1 = 3*a_0
                nc.vector.tensor_scalar_mul(s3[:, :, 1], a3[:, :, 0], 3.0)
                # s_50 = a_48 + 4*a_49 + 4*a_50
                nc.vector.scalar_tensor_tensor(
                    out=tmp[:],
                    in0=a3[:, :, 49],
                    scalar=4.0,
                    in1=a3[:, :, 48],
                    op0=mult,
                    op1=add,
                )
                nc.vector.scalar_tensor_tensor(
                    out=s3[:, :, 50],
                    in0=a3[:, :, 50],
                    scalar=4.0,
                    in1=tmp[:],
                    op0=mult,
                    op1=add,
                )

                # lt = Ln(0.25*s + 1e-35) ; lf = Ln(f + 1e-16)
                nc.scalar.activation(
                    out=ll_sb[:, 0:cols],
                    in_=s_sb[:],
                    func=mybir.ActivationFunctionType.Ln,
                    bias=eps_t[:],
                    scale=0.25,
                )
                if i == last:
                    hc = (R // 2) * ATOMS
                    nc.scalar.activation(
                        out=ll_sb[:, cols : cols + hc],
                        in_=f_sb[:, 0:hc],
                        func=mybir.ActivationFunctionType.Ln,
                        bias=eps_f[:],
                        scale=1.0,
                    )
                    nc.scalar.activation(
                        out=ll_sb[:, cols + hc : 2 * cols],
                        in_=f_sb[:, hc:cols],
                        func=mybir.ActivationFunctionType.Ln,
                        bias=eps_f[:],
                        scale=1.0,
                    )
                else:
                    nc.scalar.activation(
                        out=ll_sb[:, cols : 2 * cols],
                        in_=f_sb[:],
                        func=mybir.ActivationFunctionType.Ln,
                        bias=eps_f[:],
                        scale=1.0,
                    )

                ll2 = ll_sb[:].rearrange("p (two c) -> p two c", two=2)
                edges = [cols // 2] if i == last else []
                for c0, w in _blocks(cols, edges):
                    nc.tensor.matmul(
                        acc2[0:w, :, 0:w],
                        s_sb[:, c0 : c0 + w],
                        ll2[:, :, c0 : c0 + w],
                        start=(mm == 0),
                        stop=(mm == total_mms - 1),
                    )
                    mm += 1

            out_sb = out_pool.tile([P, 2 * MM_BLOCK], mybir.dt.float32)
            nc.vector.tensor_copy(out_sb[:], acc[:])
            nc.sync.dma_start(out=out_dram.ap(), in_=out_sb[:])

    nc.compile()
    return nc


def kernel(anchor: np.ndarray, feature: np.ndarray) -> np.ndarray:
    global _BUILT, _LAST_RESULTS
    from concourse import bass_utils

    if _BUILT is None:
        _BUILT = _build()
    nc = _BUILT

    anchor = np.ascontiguousarray(anchor, dtype=np.float32)
    feature = np.ascontiguousarray(feature, dtype=np.float32)

    in_maps = []
    for c in range(N_CORES):
        lo, hi = c * ROWS_PER_CORE, (c + 1) * ROWS_PER_CORE
        in_maps.append({"anchor": anchor[lo:hi], "feature": feature[lo:hi]})

    res = bass_utils.run_bass_kernel_spmd(
        nc,
        in_maps,
        core_ids=list(range(N_CORES)),
        trace=bool(os.environ.get("BASS_TRACE")),
    )
    _LAST_RESULTS = res

    total = 0.0
    for c in range(N_CORES):
        out = res.results[c]["out"].astype(np.float64)
        total += np.trace(out[:, :MM_BLOCK]) - np.trace(out[:, MM_BLOCK:])
    val = 0.25 * total / B_TOTAL
    return np.array(val, dtype=np.float32)


# revision 15
# speedup vs baseline: 1.2165x; 1.0456x over previous
"""KL-divergence loss kernel (C51 categorical projection + batchmean KL) for TRN2.

Math: the reference projects `anchor` through a C51 projection whose skew is a
compile-time scalar, so the projection collapses to a constant linear map:

    t[:, 0]  = 0
    t[:, 1]  = 0.75*a[:, 0]
    t[:, j]  = 0.75*a[:, j-1] + 0.25*a[:, j-2]          (2 <= j <= 49)
    t[:, 50] = 0.25*a[:, 48] + a[:, 49] + a[:, 50]

and the loss is sum(t * (log t - log(f + 1e-16))) / B  (terms with t==0 are 0).

Kernel strategy (pure data parallel over 8 cores, batch-sharded):
  s = 4t built with one wide fused scalar_tensor_tensor (s_j = 3*a_{j-1} + a_{j-2})
  lt = Ln(0.25*s + 1e-35)      [ScalarE, fused affine, bf16 out]
  lf = Ln(f + 1e-16)           [ScalarE, fused affine, bf16 out]
  lt and lf land in two halves of one fused SBUF tile; TensorE matmuls
  lhsT=s_blk against rhs spanning BOTH halves (free dim 2*w) so one PSUM
  accumulator [128, 256] collects sum(s*lt) on diag cells [j, j] and
  sum(s*lf) on [j, 128+j].  Host sums diag(lt-half) - diag(lf-half) over
  the 8 per-core results and scales by 0.25/B.  No Vector-engine subtract,
  so the per-tile V->S->V dependency chain of the naive pipeline is gone
  and DMA stays saturated.  The last tile's feature DMA + Ln are split in
  half so only ~1us of ScalarE work remains after the final transfer lands.
"""

import os
import numpy as np

B_TOTAL = 524288
ATOMS = 51
N_CORES = 8
ROWS_PER_CORE = B_TOTAL // N_CORES  # 65536
P = 128
R_SCHED = [64, 64, 64, 64, 64, 64, 64, 32, 32]
assert sum(R_SCHED) * P == ROWS_PER_CORE
MM_BLOCK = 128

_BUILT = None
_LAST_RESULTS = None


def _blocks(cols, edges):
    """128-wide matmul blocks, additionally cut at the given column edges."""
    cuts = sorted(set([0, cols] + [e for e in edges if 0 < e < cols]))
    out = []
    for lo, hi in zip(cuts[:-1], cuts[1:]):
        c = lo
        while c < hi:
            w = min(MM_BLOCK, hi - c)
            out.append((c, w))
            c += w
    return out


def _build():
    from contextlib import ExitStack

    import concourse.bacc as bacc
    import concourse.tile as tile
    from concourse import mybir

    nc = bacc.Bacc("TRN2", num_devices=N_CORES)

    a_dram = nc.dram_tensor(
        "anchor", [ROWS_PER_CORE, ATOMS], mybir.dt.float32, kind="ExternalInput"
    )
    f_dram = nc.dram_tensor(
        "feature", [ROWS_PER_CORE, ATOMS], mybir.dt.float32, kind="ExternalInput"
    )
    out_dram = nc.dram_tensor(
        "out", [P, 2 * MM_BLOCK], mybir.dt.float32, kind="ExternalOutput"
    )

    mult = mybir.AluOpType.mult
    add = mybir.AluOpType.add

    n_tiles = len(R_SCHED)
    last = n_tiles - 1

    total_mms = 0
    for i, R in enumerate(R_SCHED):
        cols = R * ATOMS
        edges = [cols // 2] if i >= last - 2 else []
        total_mms += len(_blocks(cols, edges))

    with tile.TileContext(nc) as tc:
        with ExitStack() as ctx:
            a_pool = ctx.enter_context(tc.tile_pool(name="a", bufs=3))
            f_pool = ctx.enter_context(tc.tile_pool(name="f", bufs=3))
            s_pool = ctx.enter_context(tc.tile_pool(name="s", bufs=3))
            ll_pool = ctx.enter_context(tc.tile_pool(name="ll", bufs=3))
            tmp_pool = ctx.enter_context(tc.tile_pool(name="tmp", bufs=3))
            out_pool = ctx.enter_context(tc.tile_pool(name="outp", bufs=1))
            psum_pool = ctx.enter_context(
                tc.tile_pool(name="acc", bufs=1, space="PSUM")
            )

            acc = psum_pool.tile([P, 2 * MM_BLOCK], mybir.dt.float32)
            acc2 = acc[:].rearrange("m (two c) -> m two c", two=2)

            eps_t = out_pool.tile([P, 1], mybir.dt.float32, tag="eps_t")
            eps_f = out_pool.tile([P, 1], mybir.dt.float32, tag="eps_f")
            warm = out_pool.tile([P, 1], mybir.dt.float32, tag="warm")
            nc.gpsimd.memset(eps_t[:], 1e-35)
            nc.gpsimd.memset(eps_f[:], 1e-16)
            # dummy activation: hoists the ~1.3us ACT_TABLE_LOAD off the
            # critical path (otherwise it lands right before the first real Ln)
            nc.scalar.activation(
                out=warm[:],
                in_=eps_f[:],
                func=mybir.ActivationFunctionType.Ln,
                bias=eps_f[:],
                scale=1.0,
            )

            mm = 0
            r0 = 0
            for i, R in enumerate(R_SCHED):
                cols = R * ATOMS
                a_t = (
                    a_dram.ap()[r0 : r0 + P * R, :]
                    .rearrange("(p q) m -> p (q m)", p=P)
                )
                f_t3 = (
                    f_dram.ap()[r0 : r0 + P * R, :]
                    .rearrange("(p q) m -> p q m", p=P)
                )
                r0 += P * R

                a_sb = a_pool.tile([P, cols], mybir.dt.float32)
                f_sb = f_pool.tile([P, cols], mybir.dt.float32)
                nc.sync.dma_start(out=a_sb[:], in_=a_t)
                if i >= last - 2:
                    # split the final feature transfer so Ln(f) can start on
                    # the first half while the second half is still landing
                    h = R // 2
                    nc.sync.dma_start(
                        out=f_sb[:, 0 : h * ATOMS], in_=f_t3[:, 0:h, :]
                    )
                    nc.sync.dma_start(
                        out=f_sb[:, h * ATOMS : cols], in_=f_t3[:, h:R, :]
                    )
                else:
                    nc.sync.dma_start(
                        out=f_sb[:], in_=f_t3.rearrange("p q m -> p (q m)")
                    )

                s_sb = s_pool.tile([P, cols], mybir.dt.bfloat16)
                ll_sb = ll_pool.tile([P, 2 * cols], mybir.dt.bfloat16)
                tmp = tmp_pool.tile([P, R], mybir.dt.float32)

                a3 = a_sb[:].rearrange("p (q m) -> p q m", m=ATOMS)
                s3 = s_sb[:].rearrange("p (q m) -> p q m", m=ATOMS)

                nc.gpsimd.memset(s3[:, :, 0], 0.0)
                # s_j = 3*a_{j-1} + a_{j-2} for j in 2..49
                nc.vector.scalar_tensor_tensor(
                    out=s3[:, :, 2:50],
                    in0=a3[:, :, 1:49],
                    scalar=3.0,
                    in1=a3[:, :, 0:48],
                    op0=mult,
                    op1=add,
                )
                # s_1 = 3*a_0
                nc.vector.tensor_scalar_mul(s3[:, :, 1], a3[:, :, 0], 3.0)
                # s_50 = a_48 + 4*a_49 + 4*a_50
                nc.vector.scalar_tensor_tensor(
                    out=tmp[:],
                    in0=a3[:, :, 49],
                    scalar=4.0,
                    in1=a3[:, :, 48],
                    op0=mult,
                    op1=add,
                )
                nc.vector.scalar_tensor_tensor(
                    out=s3[:, :, 50],
                    in0=a3[:, :, 50],
                    scalar=4.0,
                    in1=tmp[:],
                    op0=mult,
                    op1=add,
                )

                # lt = Ln(0.25*s + 1e-35) ; lf = Ln(f + 1e-16)
                nc.scalar.activation(
                    out=ll_sb[:, 0:cols],
                    in_=s_sb[:],
                    func=mybir.ActivationFunctionType.Ln,
                    bias=eps_t[:],
                    scale=0.25,
                )
                if i >= last - 2:
                    hc = (R // 2) * ATOMS
                    nc.scalar.activation(
                        out=ll_sb[:, cols : cols + hc],
                        in_=f_sb[:, 0:hc],
                        func=mybir.ActivationFunctionType.Ln,
                        bias=eps_f[:],
                        scale=1.0,
                    )
                    nc.scalar.activation(
                        out=ll_sb[:, cols + hc : 2 * cols],
                        in_=f_sb[:, hc:cols],
                        func=mybir.ActivationFunctionType.Ln,
                        bias=eps_f[:],
                        scale=1.0,
                    )
                else:
                    nc.scalar.activation(
                        out=ll_sb[:, cols : 2 * cols],
                        in_=f_sb[:],
                        func=mybir.ActivationFunctionType.Ln,
                        bias=eps_f[:],
                        scale=1.0,
                    )

                ll2 = ll_sb[:].rearrange("p (two c) -> p two c", two=2)
                edges = [cols // 2] if i >= last - 2 else []
                for c0, w in _blocks(cols, edges):
                    nc.tensor.matmul(
                        acc2[0:w, :, 0:w],
                        s_sb[:, c0 : c0 + w],
                        ll2[:, :, c0 : c0 + w],
                        start=(mm == 0),
                        stop=(mm == total_mms - 1),
                    )
                    mm += 1

            out_sb = out_pool.tile([P, 2 * MM_BLOCK], mybir.dt.float32)
            nc.vector.tensor_copy(out_sb[:], acc[:])
            nc.sync.dma_start(out=out_dram.ap(), in_=out_sb[:])

    nc.compile()
    return nc


def kernel(anchor: np.ndarray, feature: np.ndarray) -> np.ndarray:
    global _BUILT, _LAST_RESULTS
    from concourse import bass_utils

    if _BUILT is None:
        _BUILT = _build()
    nc = _BUILT

    anchor = np.ascontiguousarray(anchor, dtype=np.float32)
    feature = np.ascontiguousarray(feature, dtype=np.float32)

    in_maps = []
    for c in range(N_CORES):
        lo, hi = c * ROWS_PER_CORE, (c + 1) * ROWS_PER_CORE
        in_maps.append({"anchor": anchor[lo:hi], "feature": feature[lo:hi]})

    res = bass_utils.run_bass_kernel_spmd(
        nc,
        in_maps,
        core_ids=list(range(N_CORES)),
        trace=bool(os.environ.get("BASS_TRACE")),
    )
    _LAST_RESULTS = res

    total = 0.0
    for c in range(N_CORES):
        out = res.results[c]["out"].astype(np.float64)
        total += np.trace(out[:, :MM_BLOCK]) - np.trace(out[:, MM_BLOCK:])
    val = 0.25 * total / B_TOTAL
    return np.array(val, dtype=np.float32)
